# revision 41
# baseline (speedup 1.0000x reference)
"""GQA attention (B=2, T=2048, d_model=2048, 32 Q heads, 8 KV heads,
head_dim=64, RoPE, causal) on 8 Trainium2 NeuronCores -- head-parallel.

Sharding v4: core c = (batch c//4, kv-pair p=c%4). Each core projects
K/V for its ONE kv-pair (2 kv heads) over the 2048 tokens of its batch,
Q for the pair's 8 query heads, and runs fully-causal attention for all
2048 queries: exp/score element count is the exact causal half -- ~1.9x
fewer exp elements than the sequence-parallel v3 layout, which was
ACT-bound. Attention output y (512 features x 2048 tokens per core) is
repartitioned token-wise via AllToAll collectives within each 4-core
batch group; each core then applies the full output projection for its
512 tokens, emitting out^T [2048, 512] exactly like v3 (host unshard
unchanged).

The A2A is split by head slot (g0+g1 after round 1, g2 after round 2,
g3 after round 3): the first instance absorbs launch skew under two
attention rounds; the last moves only 0.5MB and its output-projection
quarter is the only serial tail. Output projection accumulates in SBUF
across the three arrivals. Q projection for g=1,2,3 is deferred into
rounds 0-2 (re-streamed x half-tiles) so the PE stays fed while ACT
grinds the exp stream."""

import sys

for _p in ("/opt/trn_rl_repo",):
    if _p not in sys.path:
        sys.path.insert(0, _p)

from contextlib import ExitStack

import numpy as np

import concourse.bass as bass  # noqa: F401
import concourse.mybir as mybir
import concourse.tile as tile
from concourse import bacc
from concourse.bass_utils import run_bass_kernel_spmd

F32 = mybir.dt.float32
BF16 = mybir.dt.bfloat16

B = 2
T = 2048
DM = 2048
HD = 64
N_CORES = 8
KT = DM // 128
NKB = T // 128
SCALE = 1.0 / float(np.sqrt(HD))


def build_gqa_hp():
    nc = bacc.Bacc(
        "TRN2", target_bir_lowering=False, debug=False, num_devices=N_CORES
    )

    xq4 = nc.dram_tensor("xq4", [128, 4, KT, 512], BF16, kind="ExternalInput")
    wq4 = nc.dram_tensor("wq4", [128, 4, KT, 128], BF16, kind="ExternalInput")
    wk4 = nc.dram_tensor("wk4", [128, KT, 128], BF16, kind="ExternalInput")
    wv4 = nc.dram_tensor("wv4", [128, KT, 128], BF16, kind="ExternalInput")
    wo4 = nc.dram_tensor("wo4", [128, 4, 16, 4, 128], BF16,
                         kind="ExternalInput")
    cosK = nc.dram_tensor("cosK", [128, T], BF16, kind="ExternalInput")
    sinK = nc.dram_tensor("sinK", [128, T], BF16, kind="ExternalInput")
    pshuf = nc.dram_tensor("pshuf", [128, 128], BF16, kind="ExternalInput")
    cmask4 = nc.dram_tensor("cmask4", [128, 4, 512], BF16,
                            kind="ExternalInput")
    selA = nc.dram_tensor("selA", [128, 4, 128], BF16, kind="ExternalInput")
    selB = nc.dram_tensor("selB", [128, 4, 128], BF16, kind="ExternalInput")
    out = nc.dram_tensor("out", [DM, 512], BF16, kind="ExternalOutput")

    # A2A buffers (8-rank; 4-rank meshes unsupported): dram part
    # d = 16*shard + s5; per-part free [rholo(8), g(n), tok(512)];
    # sbuf feature row rho = 8*s5 + rholo. Shard j carries the local
    # token block j%4, duplicated into both group halves so the same
    # SPMD program serves both batch groups; receivers pick their
    # half with a per-core 0/1 selector (bsel).
    cc_in = [
        nc.dram_tensor(f"cc_in{i}", [128, 8, n, 512], BF16)
        for i, n in ((0, 2), (1, 1), (2, 1))
    ]
    cc_out = [
        nc.dram_tensor(f"cc_out{i}", [128, 8, n, 512], BF16)
        for i, n in ((0, 2), (1, 1), (2, 1))
    ]
    CC_GS = [(0, 1), (2,), (3,)]
    GROUPS = [[0, 1, 2, 3, 4, 5, 6, 7]]
    bsel = nc.dram_tensor("bsel", [128, 2], F32, kind="ExternalInput")

    Exp = mybir.ActivationFunctionType.Exp
    Ln = mybir.ActivationFunctionType.Ln

    with tile.TileContext(nc) as tc, ExitStack() as ctx:
        PER = ctx.enter_context(tc.tile_pool(name="per", bufs=1))
        EXPP = ctx.enter_context(tc.tile_pool(name="expp", bufs=1))
        PS = ctx.enter_context(tc.tile_pool(name="ps", bufs=2, space="PSUM"))
        p1ctx = ExitStack()
        P1 = p1ctx.enter_context(tc.tile_pool(name="p1", bufs=1))

        wk_sb = PER.tile([128, KT, 128], BF16, tag="wk")
        wv_sb = PER.tile([128, KT, 128], BF16, tag="wv")
        wq_sb = PER.tile([128, 4, KT, 128], BF16, tag="wq")
        cosK_sb = PER.tile([128, T], BF16, tag="cosK")
        sinK_sb = PER.tile([128, T], BF16, tag="sinK")
        pshuf_sb = PER.tile([128, 128], BF16, tag="pshuf")
        selA_sb = PER.tile([128, 4, 128], BF16, tag="selA")
        selB_sb = PER.tile([128, 4, 128], BF16, tag="selB")
        cmask_sb = PER.tile([128, 4, 512], BF16, tag="cmask4")
        bsel_sb = PER.tile([128, 2], F32, tag="bsel")
        nc.gpsimd.dma_start(out=pshuf_sb, in_=pshuf.ap())
        nc.gpsimd.dma_start(out=bsel_sb, in_=bsel.ap())

        qrp = PER.tile([128, 4, T], BF16, tag="qrp")
        ktp = PER.tile([128, T], BF16, tag="ktp")
        vaug = PER.tile([128, NKB, 2, HD + 1], BF16, tag="vaug")
        nc.gpsimd.memset(vaug[:, :, :, HD:HD + 1], 1.0)
        yt = PER.tile([128, 4, T], BF16, tag="yt")
        denA = PER.tile([128, 2, T], BF16, tag="den")
        nc.gpsimd.memset(denA, 1.0)

        def rope_pair(dst, src_ps, cos_s, sin_s, pool):
            s_sb = pool.tile([128, 512], BF16, tag="rp_s", bufs=3,
                             name="rp_s")
            nc.vector.tensor_copy(s_sb, src_ps)
            sh_ps = PS.tile([128, 512], F32, tag="pv")
            nc.tensor.matmul(sh_ps, pshuf_sb, s_sb, start=True, stop=True)
            t1 = pool.tile([128, 512], BF16, tag="rp_t1", bufs=3, name="rp_t1")
            nc.vector.tensor_mul(t1, s_sb, cos_s)
            t2 = pool.tile([128, 512], BF16, tag="rp_t2", bufs=3, name="rp_t2")
            nc.vector.tensor_mul(t2, sh_ps, sin_s)
            nc.vector.tensor_add(dst, t1, t2)

        # ---- phase 1: K, V, Q(g=0) projections + RoPE (m-tile order)
        first = True
        for mi in range(4):
            xt = P1.tile([128, KT, 512], BF16, tag="xt", bufs=2)
            for kg in range(4):
                nc.sync.dma_start(
                    out=xt[:, 4 * kg:4 * (kg + 1), :],
                    in_=xq4.ap()[:, mi, 4 * kg:4 * (kg + 1), :],
                )
            if first:
                nc.scalar.dma_start(out=wk_sb, in_=wk4.ap())
                nc.scalar.dma_start(out=wq_sb[:, 0], in_=wq4.ap()[:, 0])
                nc.sync.dma_start(out=cosK_sb, in_=cosK.ap())
                nc.sync.dma_start(out=sinK_sb, in_=sinK.ap())
                nc.scalar.dma_start(out=wv_sb, in_=wv4.ap())
                for g in range(1, 4):
                    nc.scalar.dma_start(out=wq_sb[:, g], in_=wq4.ap()[:, g])
                nc.sync.dma_start(out=cmask_sb, in_=cmask4.ap())
                nc.sync.dma_start(out=selA_sb, in_=selA.ap())
                nc.sync.dma_start(out=selB_sb, in_=selB.ap())
                first = False
            ms = 512 * mi
            kp = PS.tile([128, 512], F32, tag="pA")
            for kt in range(KT):
                nc.tensor.matmul(
                    kp, wk_sb[:, kt, :], xt[:, kt, :],
                    start=(kt == 0), stop=(kt == KT - 1),
                )
            rope_pair(
                ktp[:, ms:ms + 512], kp,
                cosK_sb[:, ms:ms + 512], sinK_sb[:, ms:ms + 512], P1,
            )
            for g in (0, 3):
                qp = PS.tile([128, 512], F32, tag="pA")
                for kt in range(KT):
                    nc.tensor.matmul(
                        qp, wq_sb[:, g, kt, :], xt[:, kt, :],
                        start=(kt == 0), stop=(kt == KT - 1),
                    )
                rope_pair(
                    qrp[:, g, ms:ms + 512], qp,
                    cosK_sb[:, ms:ms + 512], sinK_sb[:, ms:ms + 512], P1,
                )
            for j in range(4):
                kb = 4 * mi + j
                vp = PS.tile([128, 128], F32, tag="pv")
                for kt in range(KT):
                    nc.tensor.matmul(
                        vp, xt[:, kt, 128 * j:128 * (j + 1)], wv_sb[:, kt, :],
                        start=(kt == 0), stop=(kt == KT - 1),
                    )
                nc.scalar.copy(vaug[:, kb, :, 0:HD], vp)
        p1ctx.close()

        p3ctx = ExitStack()
        PX = p3ctx.enter_context(tc.tile_pool(name="px", bufs=1))
        wo_sb = PX.tile([128, 2, 16, 4, 128], BF16, tag="wo")
        oacc = PX.tile([128, 16, 512], BF16, tag="oacc")
        yF = PX.tile([128, 4, 4, 512], BF16, tag="yF")

        # ---- deferred Q projection (g=1..3 inside rounds g-1)
        xh_q = []

        def q_prefetch(g):
            for mi in range(4):
                for h in range(2):
                    xh = PX.tile([128, 8, 512], BF16, tag="xh", bufs=2,
                                 name="xh")
                    nc.sync.dma_start(
                        out=xh, in_=xq4.ap()[:, mi, 8 * h:8 * h + 8, :]
                    )
                    xh_q.append((g, mi, h, xh))

        def q_proj_deferred(n_mi):
            for _ in range(n_mi):
                qp = PS.tile([128, 512], F32, tag="pA", name="qpd")
                for hh in range(2):
                    g, mi, h, xh = xh_q.pop(0)
                    for kt in range(8):
                        nc.tensor.matmul(
                            qp, wq_sb[:, g, 8 * h + kt, :], xh[:, kt, :],
                            start=(h == 0 and kt == 0),
                            stop=(h == 1 and kt == 7),
                        )
                ms = 512 * mi
                rope_pair(
                    qrp[:, g, ms:ms + 512], qp,
                    cosK_sb[:, ms:ms + 512], sinK_sb[:, ms:ms + 512], PX,
                )

        def normalize(g, qp):
            # per 1024-token pair of query chunks: halves the ACT-queue
            # injections that bubble the exp stream at chunk boundaries
            qs = 1024 * qp
            base = 32 * g
            lnA = PX.tile([32, 2, 1024], F32, tag="lnA", bufs=1)
            nc.scalar.activation(lnA, denA[base:base + 32, :, qs:qs + 1024],
                                 Ln)
            recipT = PX.tile([128, 2, 1024], BF16, tag="recipT", bufs=1)
            nc.scalar.activation(
                recipT[base:base + 32, :, :], lnA, Exp, scale=-1.0
            )
            kw = {"tile_position": (96, 0)} if g == 3 else {}
            for h in range(2):
                hs = 512 * h
                rb_ps = PS.tile([128, 512], F32, tag="pA")
                nc.tensor.matmul(
                    rb_ps, selA_sb[base:base + 32, g, :],
                    recipT[base:base + 32, 0, hs:hs + 512],
                    start=True, stop=False, **kw,
                )
                nc.tensor.matmul(
                    rb_ps, selB_sb[base:base + 32, g, :],
                    recipT[base:base + 32, 1, hs:hs + 512],
                    start=False, stop=True, **kw,
                )
                rb_sb = PX.tile([128, 512], BF16, tag="rb", bufs=2)
                nc.vector.tensor_copy(rb_sb, rb_ps)
                nc.vector.tensor_mul(
                    yt[:, g, qs + hs:qs + hs + 512],
                    yt[:, g, qs + hs:qs + hs + 512], rb_sb
                )

        def attention_round(g, qcs, filler=None):
            for qc in qcs:
                qs = 512 * qc
                pv = [
                    PS.tile([HD + 1, 512], F32, tag="pv",
                            name=f"pv{g}{qc}{hh}")
                    for hh in range(2)
                ]
                nkb = 4 * qc + 4
                for kb in range(nkb):
                    ql = 128 * max(0, kb - 4 * qc)
                    s2 = PS.tile([128, 2, 512], F32, tag="s2", name="s2")
                    for hh in range(2):
                        nc.tensor.matmul(
                            s2[:, hh, ql:],
                            ktp[64 * hh:64 * (hh + 1),
                                128 * kb:128 * (kb + 1)],
                            qrp[64 * hh:64 * (hh + 1), g, qs + ql:qs + 512],
                            start=True, stop=True,
                            tile_position=(64 * hh, 0),
                        )
                    e_sb = EXPP.tile([128, 2, 512], BF16, tag="e_sb", bufs=6)
                    nc.scalar.activation(
                        e_sb[:, :, ql:], s2[:, :, ql:], Exp, scale=SCALE,
                    )
                    di = kb - 4 * qc
                    if di >= 0:
                        for hh in range(2):
                            nc.vector.tensor_mul(
                                e_sb[:, hh, ql:], e_sb[:, hh, ql:],
                                cmask_sb[:, di, ql:],
                            )
                    for hh in range(2):
                        nc.tensor.matmul(
                            pv[hh][:, ql:], vaug[:, kb, hh, :],
                            e_sb[:, hh, ql:],
                            start=(kb == 0), stop=(kb == nkb - 1),
                            skip_group_check=(ql > 0),
                        )
                    if filler and kb % 2 == 1:
                        filler.pop(0)()
                for hh in range(2):
                    nc.vector.tensor_copy(
                        denA[32 * g:32 * g + 1, hh, qs:qs + 512],
                        pv[hh][HD:HD + 1, :],
                    )
                    nc.vector.tensor_copy(
                        yt[64 * hh:64 * (hh + 1), g, qs:qs + 512],
                        pv[hh][0:HD, :],
                    )
                if qc % 2 == 1:
                    normalize(g, qc // 2)

        # ---- collectives (manual sems: DRAM deps aren't tile-tracked)

        def write_cc(idx, ts):
            gs = CC_GS[idx]
            for t in ts:
                for rep in (t, t + 4):
                    nc.gpsimd.dma_start(
                        out=cc_in[idx].ap()[16 * rep:16 * (rep + 1)],
                        in_=yt[:, gs[0]:gs[-1] + 1, 512 * t:512 * (t + 1)],
                    )

        def emit_a2a(idx, ts=(0, 1, 2, 3)):
            write_cc(idx, ts)
            nc.gpsimd.collective_compute(
                "AllToAll",
                mybir.AluOpType.bypass,
                replica_groups=GROUPS,
                ins=[cc_in[idx].ap().opt()],
                outs=[cc_out[idx].ap().opt()],
            )

        def load_wo_slab(g):
            # ACT queue: idle in the tail; gpsimd carries the collective
            # chain and would serialize these behind the A2A launches
            nc.scalar.dma_start(out=wo_sb[:, g % 2], in_=wo4.ap()[:, g])

        def fetch_yF(idx):
            gs = CC_GS[idx]
            for s in range(4):
                for gi, g in enumerate(gs):
                    ya = PX.tile([128, 512], BF16, tag="ya", bufs=2,
                                 name="ya")
                    yb = PX.tile([128, 512], BF16, tag="yb", bufs=2,
                                 name="yb")
                    nc.sync.dma_start(
                        out=ya,
                        in_=cc_out[idx].ap()[16 * s:16 * (s + 1), :, gi, :],
                    )
                    nc.sync.dma_start(
                        out=yb,
                        in_=cc_out[idx].ap()[16 * (s + 4):16 * (s + 5), :,
                                             gi, :],
                    )
                    nc.vector.tensor_scalar_mul(ya, ya, bsel_sb[:, 0:1])
                    nc.vector.tensor_scalar_mul(yb, yb, bsel_sb[:, 1:2])
                    nc.vector.tensor_add(yF[:, g, s, :], ya, yb)

        def wo_chain_n(g, n, mode):
            op = PS.tile([128, 512], F32, tag="pA")
            for s in range(4):
                nc.tensor.matmul(
                    op, wo_sb[:, g % 2, n, s, :], yF[:, g, s, :],
                    start=(s == 0), stop=(s == 3),
                )
            if mode == 0:
                nc.vector.tensor_copy(oacc[:, n, :], op)
            elif mode == 1:
                nc.vector.tensor_add(oacc[:, n, :], oacc[:, n, :], op)
            else:
                ot = PX.tile([128, 512], BF16, tag="ot", bufs=2)
                nc.vector.tensor_add(ot, oacc[:, n, :], op)
                nc.sync.dma_start(
                    out=out.ap()[128 * n:128 * (n + 1), :], in_=ot
                )

        def wo_chain_n23(n):
            op = PS.tile([128, 512], F32, tag="pA")
            for gi, g in enumerate((2, 3)):
                for s in range(4):
                    nc.tensor.matmul(
                        op, wo_sb[:, g % 2, n, s, :], yF[:, g, s, :],
                        start=(gi == 0 and s == 0), stop=(gi == 1 and s == 3),
                    )
            ot = PX.tile([128, 512], BF16, tag="ot", bufs=2)
            nc.vector.tensor_add(ot, oacc[:, n, :], op)
            nc.sync.dma_start(
                out=out.ap()[128 * n:128 * (n + 1), :], in_=ot
            )

        def wo_chain(g, mode):
            """One output-projection pass: head-slot g's 4 F-tiles into
            all 16 out-row blocks. mode 0 = init oacc, 1 = accumulate,
            2 = final add + store."""
            for n in range(16):
                wo_chain_n(g, n, mode)

        # ---- rounds with deferred work woven in
        q_prefetch(1)
        for qc in range(4):
            attention_round(0, (qc,))
            q_proj_deferred(1)
        q_prefetch(2)
        for qc in range(4):
            attention_round(1, (qc,))
            q_proj_deferred(1)
        emit_a2a(0)
        attention_round(2, (0, 1))
        attention_round(2, (2, 3))
        emit_a2a(1)
        load_wo_slab(0)
        load_wo_slab(1)
        attention_round(3, (0, 1))
        write_cc(2, (0, 1))
        fetch_yF(0)
        wo01_fill = [
            (lambda n=n: wo_chain_n(0, n, 0)) for n in range(16)
        ] + [
            (lambda n=n: wo_chain_n(1, n, 1)) for n in range(16)
        ]
        attention_round(3, (2, 3), filler=wo01_fill)
        emit_a2a(2, ts=(2, 3))
        for th in wo01_fill:
            th()
        load_wo_slab(2)
        fetch_yF(1)
        wo_chain(2, 1)
        load_wo_slab(3)
        fetch_yF(2)
        wo_chain(3, 2)
        p3ctx.close()

    nc.finalize()
    return nc


def make_inputs(x, cos, sin, wq, wk, wv, wo):
    """Host-side sharding/layout prep. Returns in_maps for the 8 cores."""
    import ml_dtypes

    bf = ml_dtypes.bfloat16

    def b(arr):
        return np.ascontiguousarray(np.asarray(arr, dtype=bf))

    sgn = np.concatenate(
        [-np.ones(32, np.float32), np.ones(32, np.float32)]
    )
    pshuf = np.zeros((128, 128), np.float32)
    for m in range(128):
        pshuf[64 * (m // 64) + (m % 64 + 32) % 64, m] = 1.0
    selA = np.zeros((128, 4, 128), np.float32)
    selB = np.zeros((128, 4, 128), np.float32)
    for g in range(4):
        selA[32 * g, g, 0:64] = 1.0
        selB[32 * g, g, 64:128] = 1.0
    p = np.arange(128)[:, None]
    q = np.arange(512)[None, :]
    cmask4 = np.stack(
        [(128 * r + p <= q).astype(np.float32) for r in range(4)]
    ).transpose(1, 0, 2)  # [128, 4, 512]
    pshuf_b, selA_b, selB_b, cmask_b = b(pshuf), b(selA), b(selB), b(cmask4)

    cosK_b = b(np.tile(np.asarray(cos, np.float32).T, (2, 1)))
    sinK_b = b(np.tile(np.asarray(sin, np.float32).T * sgn[:, None], (2, 1)))

    wqT = np.asarray(wq, np.float32).T    # [in 2048, out 2048]
    wkT = np.asarray(wk, np.float32).T    # [in 2048, out 512]
    wvT = np.asarray(wv, np.float32).T
    woM = np.asarray(wo, np.float32)      # [out 2048, in 2048]

    # global repartitioned feature f = 512 s + 128 g + 64 hh + d
    # <-> model head 8 s + 4 hh + g, dim d
    colmap4 = np.array([
        64 * (8 * s + 4 * hh + g) + d
        for s in range(4) for g in range(4) for hh in range(2)
        for d in range(64)
    ])
    W = woM[:, colmap4].T  # [in(f) 2048, out 2048]
    wo4 = b(
        W.reshape(4, 4, 128, 16, 128).transpose(2, 1, 3, 0, 4)
    )  # [rho_in 128, g 4, n 16, s 4, rho_out 128]

    in_maps = []
    for c in range(N_CORES):
        bc, pp = c // 4, c % 4
        xbT = np.asarray(x[bc], np.float32).T
        xq4 = b(xbT.reshape(KT, 128, 4, 512).transpose(1, 2, 0, 3))
        # wq: pair pp, slot g holds heads 8pp+4hh+g at rows 64hh+d
        qcols = np.array([
            64 * (8 * pp + 4 * hh + g) + d
            for g in range(4) for hh in range(2) for d in range(64)
        ])
        wq4 = b(
            wqT[:, qcols].reshape(KT, 128, 4, 128).transpose(1, 2, 0, 3)
        )
        kcols = np.array([
            64 * (2 * pp + hh) + d for hh in range(2) for d in range(64)
        ])
        wk4 = b(wkT[:, kcols].reshape(KT, 128, 128).transpose(1, 0, 2))
        wv4 = b(wvT[:, kcols].reshape(KT, 128, 128).transpose(1, 0, 2))
        bsel_c = np.zeros((128, 2), np.float32)
        bsel_c[:, 0 if bc == 0 else 1] = 1.0
        in_maps.append(
            {
                "xq4": xq4, "wq4": wq4, "wk4": wk4, "wv4": wv4, "wo4": wo4,
                "cosK": cosK_b, "sinK": sinK_b, "pshuf": pshuf_b,
                "cmask4": cmask_b, "selA": selA_b, "selB": selB_b,
                "bsel": bsel_c,
            }
        )
    return in_maps


_NC_CACHE = {}


def get_nc():
    if "hp" not in _NC_CACHE:
        _NC_CACHE["hp"] = build_gqa_hp()
    return _NC_CACHE["hp"]


def kernel(x, cos, sin, wq, wk, wv, wo, _trace=False):
    x = np.asarray(x, np.float32)
    nc = get_nc()
    in_maps = make_inputs(
        x,
        np.asarray(cos, np.float32),
        np.asarray(sin, np.float32),
        np.asarray(wq, np.float32),
        np.asarray(wk, np.float32),
        np.asarray(wv, np.float32),
        np.asarray(wo, np.float32),
    )
    res = run_bass_kernel_spmd(nc, in_maps, list(range(N_CORES)), trace=_trace)
    # core c returns out^T [2048, 512] (bf16) for batch c//4, tokens
    # [512*(c%4), 512*(c%4)+512)
    full = np.empty((B, T, DM), np.float32)
    for c in range(N_CORES):
        bc, pp = c // 4, c % 4
        full[bc, 512 * pp:512 * (pp + 1), :] = np.asarray(
            res.results[c]["out"], np.float32
        ).T
    if _trace:
        return full, res
    return full


# revision 42
# speedup vs baseline: 1.0665x; 1.0665x over previous
"""GQA attention (B=2, T=2048, d_model=2048, 32 Q heads, 8 KV heads,
head_dim=64, RoPE, causal) on 8 Trainium2 NeuronCores -- head-parallel.

Sharding v4: core c = (batch c//4, kv-pair p=c%4). Each core projects
K/V for its ONE kv-pair (2 kv heads) over the 2048 tokens of its batch,
Q for the pair's 8 query heads, and runs fully-causal attention for all
2048 queries: exp/score element count is the exact causal half -- ~1.9x
fewer exp elements than the sequence-parallel v3 layout, which was
ACT-bound. Attention output y (512 features x 2048 tokens per core) is
repartitioned token-wise via AllToAll collectives within each 4-core
batch group; each core then applies the full output projection for its
512 tokens, emitting out^T [2048, 512] exactly like v3 (host unshard
unchanged).

The A2A is split by head slot (g0+g1 after round 1, g2 after round 2,
g3 after round 3): the first instance absorbs launch skew under two
attention rounds; the last moves only 0.5MB and its output-projection
quarter is the only serial tail. Output projection accumulates in SBUF
across the three arrivals. Q projection for g=1,2,3 is deferred into
rounds 0-2 (re-streamed x half-tiles) so the PE stays fed while ACT
grinds the exp stream."""

import sys

for _p in ("/opt/trn_rl_repo",):
    if _p not in sys.path:
        sys.path.insert(0, _p)

from contextlib import ExitStack

import numpy as np

import concourse.bass as bass  # noqa: F401
import concourse.mybir as mybir
import concourse.tile as tile
from concourse import bacc
from concourse.bass_utils import run_bass_kernel_spmd

F32 = mybir.dt.float32
BF16 = mybir.dt.bfloat16

B = 2
T = 2048
DM = 2048
HD = 64
N_CORES = 8
KT = DM // 128
NKB = T // 128
SCALE = 1.0 / float(np.sqrt(HD))


def build_gqa_hp():
    nc = bacc.Bacc(
        "TRN2", target_bir_lowering=False, debug=False, num_devices=N_CORES
    )

    xq4 = nc.dram_tensor("xq4", [128, 4, KT, 512], BF16, kind="ExternalInput")
    wq4 = nc.dram_tensor("wq4", [128, 4, KT, 128], BF16, kind="ExternalInput")
    wk4 = nc.dram_tensor("wk4", [128, KT, 128], BF16, kind="ExternalInput")
    wv4 = nc.dram_tensor("wv4", [128, KT, 128], BF16, kind="ExternalInput")
    wo4 = nc.dram_tensor("wo4", [128, 4, 16, 4, 128], BF16,
                         kind="ExternalInput")
    cosK = nc.dram_tensor("cosK", [128, T], BF16, kind="ExternalInput")
    sinK = nc.dram_tensor("sinK", [128, T], BF16, kind="ExternalInput")
    pshuf = nc.dram_tensor("pshuf", [128, 128], BF16, kind="ExternalInput")
    cmask4 = nc.dram_tensor("cmask4", [128, 4, 512], BF16,
                            kind="ExternalInput")
    selA = nc.dram_tensor("selA", [128, 4, 128], BF16, kind="ExternalInput")
    selB = nc.dram_tensor("selB", [128, 4, 128], BF16, kind="ExternalInput")
    out = nc.dram_tensor("out", [DM, 512], BF16, kind="ExternalOutput")

    # A2A buffers (8-rank; 4-rank meshes unsupported): dram part
    # d = 16*shard + s5; per-part free [rholo(8), g(n), tok(512)];
    # sbuf feature row rho = 8*s5 + rholo. Shard j carries the local
    # token block j%4, duplicated into both group halves so the same
    # SPMD program serves both batch groups; receivers pick their
    # half with a per-core 0/1 selector (bsel).
    cc_in = [
        nc.dram_tensor(f"cc_in{i}", [128, 8, 1, 512], BF16)
        for i in range(4)
    ]
    cc_out = [
        nc.dram_tensor(f"cc_out{i}", [128, 8, 1, 512], BF16)
        for i in range(4)
    ]
    CC_GS = [(0,), (1,), (2,), (3,)]
    GROUPS = [[0, 1, 2, 3, 4, 5, 6, 7]]
    bsel = nc.dram_tensor("bsel", [128, 2], F32, kind="ExternalInput")

    Exp = mybir.ActivationFunctionType.Exp
    Ln = mybir.ActivationFunctionType.Ln

    with tile.TileContext(nc) as tc, ExitStack() as ctx:
        PER = ctx.enter_context(tc.tile_pool(name="per", bufs=1))
        EXPP = ctx.enter_context(tc.tile_pool(name="expp", bufs=1))
        PS = ctx.enter_context(tc.tile_pool(name="ps", bufs=2, space="PSUM"))
        p1ctx = ExitStack()
        P1 = p1ctx.enter_context(tc.tile_pool(name="p1", bufs=1))

        wk_sb = PER.tile([128, KT, 128], BF16, tag="wk")
        wv_sb = PER.tile([128, KT, 128], BF16, tag="wv")
        wq_sb = PER.tile([128, 4, KT, 128], BF16, tag="wq")
        cosK_sb = PER.tile([128, T], BF16, tag="cosK")
        sinK_sb = PER.tile([128, T], BF16, tag="sinK")
        pshuf_sb = PER.tile([128, 128], BF16, tag="pshuf")
        selA_sb = PER.tile([128, 4, 128], BF16, tag="selA")
        selB_sb = PER.tile([128, 4, 128], BF16, tag="selB")
        cmask_sb = PER.tile([128, 4, 512], BF16, tag="cmask4")
        bsel_sb = PER.tile([128, 2], F32, tag="bsel")
        nc.gpsimd.dma_start(out=pshuf_sb, in_=pshuf.ap())
        nc.gpsimd.dma_start(out=bsel_sb, in_=bsel.ap())

        qrp = PER.tile([128, 4, T], BF16, tag="qrp")
        ktp = PER.tile([128, T], BF16, tag="ktp")
        vaug = PER.tile([128, NKB, 2, HD + 1], BF16, tag="vaug")
        nc.gpsimd.memset(vaug[:, :, :, HD:HD + 1], 1.0)
        yt = PER.tile([128, 4, T], BF16, tag="yt")
        denA = PER.tile([128, 2, T], BF16, tag="den")
        nc.gpsimd.memset(denA, 1.0)

        def rope_pair(dst, src_ps, cos_s, sin_s, pool):
            s_sb = pool.tile([128, 512], BF16, tag="rp_s", bufs=3,
                             name="rp_s")
            nc.vector.tensor_copy(s_sb, src_ps)
            sh_ps = PS.tile([128, 512], F32, tag="pv")
            nc.tensor.matmul(sh_ps, pshuf_sb, s_sb, start=True, stop=True)
            t1 = pool.tile([128, 512], BF16, tag="rp_t1", bufs=3, name="rp_t1")
            nc.vector.tensor_mul(t1, s_sb, cos_s)
            t2 = pool.tile([128, 512], BF16, tag="rp_t2", bufs=3, name="rp_t2")
            nc.vector.tensor_mul(t2, sh_ps, sin_s)
            nc.vector.tensor_add(dst, t1, t2)

        # ---- phase 1: K, V, Q(g=0) projections + RoPE (m-tile order)
        first = True
        for mi in range(4):
            xt = P1.tile([128, KT, 512], BF16, tag="xt", bufs=2)
            for kg in range(4):
                nc.sync.dma_start(
                    out=xt[:, 4 * kg:4 * (kg + 1), :],
                    in_=xq4.ap()[:, mi, 4 * kg:4 * (kg + 1), :],
                )
            if first:
                nc.scalar.dma_start(out=wk_sb, in_=wk4.ap())
                nc.scalar.dma_start(out=wq_sb[:, 0], in_=wq4.ap()[:, 0])
                nc.sync.dma_start(out=cosK_sb, in_=cosK.ap())
                nc.sync.dma_start(out=sinK_sb, in_=sinK.ap())
                nc.scalar.dma_start(out=wv_sb, in_=wv4.ap())
                for g in range(1, 4):
                    nc.scalar.dma_start(out=wq_sb[:, g], in_=wq4.ap()[:, g])
                nc.sync.dma_start(out=cmask_sb, in_=cmask4.ap())
                nc.sync.dma_start(out=selA_sb, in_=selA.ap())
                nc.sync.dma_start(out=selB_sb, in_=selB.ap())
                first = False
            ms = 512 * mi
            kp = PS.tile([128, 512], F32, tag="pA")
            for kt in range(KT):
                nc.tensor.matmul(
                    kp, wk_sb[:, kt, :], xt[:, kt, :],
                    start=(kt == 0), stop=(kt == KT - 1),
                )
            rope_pair(
                ktp[:, ms:ms + 512], kp,
                cosK_sb[:, ms:ms + 512], sinK_sb[:, ms:ms + 512], P1,
            )
            for g in (0, 3):
                qp = PS.tile([128, 512], F32, tag="pA")
                for kt in range(KT):
                    nc.tensor.matmul(
                        qp, wq_sb[:, g, kt, :], xt[:, kt, :],
                        start=(kt == 0), stop=(kt == KT - 1),
                    )
                rope_pair(
                    qrp[:, g, ms:ms + 512], qp,
                    cosK_sb[:, ms:ms + 512], sinK_sb[:, ms:ms + 512], P1,
                )
            for j in range(4):
                kb = 4 * mi + j
                vp = PS.tile([128, 128], F32, tag="pv")
                for kt in range(KT):
                    nc.tensor.matmul(
                        vp, xt[:, kt, 128 * j:128 * (j + 1)], wv_sb[:, kt, :],
                        start=(kt == 0), stop=(kt == KT - 1),
                    )
                nc.scalar.copy(vaug[:, kb, :, 0:HD], vp)
        p1ctx.close()

        p3ctx = ExitStack()
        PX = p3ctx.enter_context(tc.tile_pool(name="px", bufs=1))
        wo_sb = PX.tile([128, 2, 16, 4, 128], BF16, tag="wo")
        oacc = PX.tile([128, 16, 512], BF16, tag="oacc")
        yF = PX.tile([128, 4, 4, 512], BF16, tag="yF")

        # ---- deferred Q projection (g=1..3 inside rounds g-1)
        xh_q = []

        def q_prefetch(g):
            for mi in range(4):
                for h in range(2):
                    xh = PX.tile([128, 8, 512], BF16, tag="xh", bufs=2,
                                 name="xh")
                    nc.sync.dma_start(
                        out=xh, in_=xq4.ap()[:, mi, 8 * h:8 * h + 8, :]
                    )
                    xh_q.append((g, mi, h, xh))

        def q_proj_deferred(n_mi):
            for _ in range(n_mi):
                qp = PS.tile([128, 512], F32, tag="pA", name="qpd")
                for hh in range(2):
                    g, mi, h, xh = xh_q.pop(0)
                    for kt in range(8):
                        nc.tensor.matmul(
                            qp, wq_sb[:, g, 8 * h + kt, :], xh[:, kt, :],
                            start=(h == 0 and kt == 0),
                            stop=(h == 1 and kt == 7),
                        )
                ms = 512 * mi
                rope_pair(
                    qrp[:, g, ms:ms + 512], qp,
                    cosK_sb[:, ms:ms + 512], sinK_sb[:, ms:ms + 512], PX,
                )

        def normalize(g, qp):
            # broadcast den to 128 partitions FIRST (the sel matmuls),
            # then one full-width DVE reciprocal: no ACT involvement at
            # all -- the old ln/exp pair cost two ACT table-set switches
            # (~2.6us) plus 2x2us of 32-partition ACT per call, right on
            # the round-boundary critical path
            qs = 1024 * qp
            base = 32 * g
            kw = {"tile_position": (96, 0)} if g == 3 else {}
            for h in range(2):
                hs = 512 * h
                rb_ps = PS.tile([128, 512], F32, tag="pA")
                nc.tensor.matmul(
                    rb_ps, selA_sb[base:base + 32, g, :],
                    denA[base:base + 32, 0, qs + hs:qs + hs + 512],
                    start=True, stop=False, **kw,
                )
                nc.tensor.matmul(
                    rb_ps, selB_sb[base:base + 32, g, :],
                    denA[base:base + 32, 1, qs + hs:qs + hs + 512],
                    start=False, stop=True, **kw,
                )
                rcp = PX.tile([128, 512], F32, tag="rcp", bufs=2)
                nc.vector.reciprocal(rcp, rb_ps)
                nc.vector.tensor_mul(
                    yt[:, g, qs + hs:qs + hs + 512],
                    yt[:, g, qs + hs:qs + hs + 512], rcp
                )

        def attention_round(g, qcs, filler=None):
            for qc in qcs:
                qs = 512 * qc
                pv = [
                    PS.tile([HD + 1, 512], F32, tag="pv",
                            name=f"pv{g}{qc}{hh}")
                    for hh in range(2)
                ]
                nkb = 4 * qc + 4
                for kb in range(nkb):
                    ql = 128 * max(0, kb - 4 * qc)
                    s2 = PS.tile([128, 2, 512], F32, tag="s2", name="s2")
                    for hh in range(2):
                        nc.tensor.matmul(
                            s2[:, hh, ql:],
                            ktp[64 * hh:64 * (hh + 1),
                                128 * kb:128 * (kb + 1)],
                            qrp[64 * hh:64 * (hh + 1), g, qs + ql:qs + 512],
                            start=True, stop=True,
                            tile_position=(64 * hh, 0),
                        )
                    e_sb = EXPP.tile([128, 2, 512], BF16, tag="e_sb", bufs=6)
                    nc.scalar.activation(
                        e_sb[:, :, ql:], s2[:, :, ql:], Exp, scale=SCALE,
                    )
                    di = kb - 4 * qc
                    if di >= 0:
                        for hh in range(2):
                            nc.vector.tensor_mul(
                                e_sb[:, hh, ql:], e_sb[:, hh, ql:],
                                cmask_sb[:, di, ql:],
                            )
                    for hh in range(2):
                        nc.tensor.matmul(
                            pv[hh][:, ql:], vaug[:, kb, hh, :],
                            e_sb[:, hh, ql:],
                            start=(kb == 0), stop=(kb == nkb - 1),
                            skip_group_check=(ql > 0),
                        )
                    if filler and kb % 2 == 1:
                        filler.pop(0)()
                for hh in range(2):
                    nc.vector.tensor_copy(
                        denA[32 * g:32 * g + 1, hh, qs:qs + 512],
                        pv[hh][HD:HD + 1, :],
                    )
                    nc.vector.tensor_copy(
                        yt[64 * hh:64 * (hh + 1), g, qs:qs + 512],
                        pv[hh][0:HD, :],
                    )
                if qc % 2 == 1:
                    normalize(g, qc // 2)

        # ---- collectives (manual sems: DRAM deps aren't tile-tracked)

        def write_cc(idx, ts):
            gs = CC_GS[idx]
            for t in ts:
                for rep in (t, t + 4):
                    nc.gpsimd.dma_start(
                        out=cc_in[idx].ap()[16 * rep:16 * (rep + 1)],
                        in_=yt[:, gs[0]:gs[-1] + 1, 512 * t:512 * (t + 1)],
                    )

        def emit_a2a(idx, ts=(0, 1, 2, 3)):
            write_cc(idx, ts)
            nc.gpsimd.collective_compute(
                "AllToAll",
                mybir.AluOpType.bypass,
                replica_groups=GROUPS,
                ins=[cc_in[idx].ap().opt()],
                outs=[cc_out[idx].ap().opt()],
            )

        def load_wo_slab(g):
            # ACT queue: idle in the tail; gpsimd carries the collective
            # chain and would serialize these behind the A2A launches
            nc.scalar.dma_start(out=wo_sb[:, g % 2], in_=wo4.ap()[:, g])

        def fetch_yF(idx):
            gs = CC_GS[idx]
            for s in range(4):
                for gi, g in enumerate(gs):
                    ya = PX.tile([128, 512], BF16, tag="ya", bufs=2,
                                 name="ya")
                    yb = PX.tile([128, 512], BF16, tag="yb", bufs=2,
                                 name="yb")
                    nc.sync.dma_start(
                        out=ya,
                        in_=cc_out[idx].ap()[16 * s:16 * (s + 1), :, gi, :],
                    )
                    nc.sync.dma_start(
                        out=yb,
                        in_=cc_out[idx].ap()[16 * (s + 4):16 * (s + 5), :,
                                             gi, :],
                    )
                    nc.vector.tensor_scalar_mul(ya, ya, bsel_sb[:, 0:1])
                    nc.vector.tensor_scalar_mul(yb, yb, bsel_sb[:, 1:2])
                    nc.vector.tensor_add(yF[:, g, s, :], ya, yb)

        def wo_chain_n(g, n, mode):
            op = PS.tile([128, 512], F32, tag="pA")
            for s in range(4):
                nc.tensor.matmul(
                    op, wo_sb[:, g % 2, n, s, :], yF[:, g, s, :],
                    start=(s == 0), stop=(s == 3),
                )
            if mode == 0:
                nc.vector.tensor_copy(oacc[:, n, :], op)
            elif mode == 1:
                nc.vector.tensor_add(oacc[:, n, :], oacc[:, n, :], op)
            else:
                ot = PX.tile([128, 512], BF16, tag="ot", bufs=2)
                nc.vector.tensor_add(ot, oacc[:, n, :], op)
                nc.sync.dma_start(
                    out=out.ap()[128 * n:128 * (n + 1), :], in_=ot
                )

        def wo_chain_n23(n):
            op = PS.tile([128, 512], F32, tag="pA")
            for gi, g in enumerate((2, 3)):
                for s in range(4):
                    nc.tensor.matmul(
                        op, wo_sb[:, g % 2, n, s, :], yF[:, g, s, :],
                        start=(gi == 0 and s == 0), stop=(gi == 1 and s == 3),
                    )
            ot = PX.tile([128, 512], BF16, tag="ot", bufs=2)
            nc.vector.tensor_add(ot, oacc[:, n, :], op)
            nc.sync.dma_start(
                out=out.ap()[128 * n:128 * (n + 1), :], in_=ot
            )

        def wo_chain(g, mode):
            """One output-projection pass: head-slot g's 4 F-tiles into
            all 16 out-row blocks. mode 0 = init oacc, 1 = accumulate,
            2 = final add + store."""
            for n in range(16):
                wo_chain_n(g, n, mode)

        # ---- rounds with deferred work woven in
        q_prefetch(1)
        for qc in range(4):
            attention_round(0, (qc,))
            q_proj_deferred(1)
        emit_a2a(0)
        q_prefetch(2)
        for qc in range(4):
            attention_round(1, (qc,))
            q_proj_deferred(1)
        emit_a2a(1)
        attention_round(2, (0, 1))
        attention_round(2, (2, 3))
        emit_a2a(2)
        load_wo_slab(0)
        load_wo_slab(1)
        attention_round(3, (0, 1))
        write_cc(3, (0, 1))
        fetch_yF(0)
        fetch_yF(1)
        wo01_fill = [
            (lambda n=n: wo_chain_n(0, n, 0)) for n in range(16)
        ] + [
            (lambda n=n: wo_chain_n(1, n, 1)) for n in range(16)
        ]
        attention_round(3, (2, 3), filler=wo01_fill)
        emit_a2a(3, ts=(2, 3))
        for th in wo01_fill:
            th()
        load_wo_slab(2)
        fetch_yF(2)
        wo_chain(2, 1)
        load_wo_slab(3)
        fetch_yF(3)
        wo_chain(3, 2)
        p3ctx.close()

    nc.finalize()
    return nc


def make_inputs(x, cos, sin, wq, wk, wv, wo):
    """Host-side sharding/layout prep. Returns in_maps for the 8 cores."""
    import ml_dtypes

    bf = ml_dtypes.bfloat16

    def b(arr):
        return np.ascontiguousarray(np.asarray(arr, dtype=bf))

    sgn = np.concatenate(
        [-np.ones(32, np.float32), np.ones(32, np.float32)]
    )
    pshuf = np.zeros((128, 128), np.float32)
    for m in range(128):
        pshuf[64 * (m // 64) + (m % 64 + 32) % 64, m] = 1.0
    selA = np.zeros((128, 4, 128), np.float32)
    selB = np.zeros((128, 4, 128), np.float32)
    for g in range(4):
        selA[32 * g, g, 0:64] = 1.0
        selB[32 * g, g, 64:128] = 1.0
    p = np.arange(128)[:, None]
    q = np.arange(512)[None, :]
    cmask4 = np.stack(
        [(128 * r + p <= q).astype(np.float32) for r in range(4)]
    ).transpose(1, 0, 2)  # [128, 4, 512]
    pshuf_b, selA_b, selB_b, cmask_b = b(pshuf), b(selA), b(selB), b(cmask4)

    cosK_b = b(np.tile(np.asarray(cos, np.float32).T, (2, 1)))
    sinK_b = b(np.tile(np.asarray(sin, np.float32).T * sgn[:, None], (2, 1)))

    wqT = np.asarray(wq, np.float32).T    # [in 2048, out 2048]
    wkT = np.asarray(wk, np.float32).T    # [in 2048, out 512]
    wvT = np.asarray(wv, np.float32).T
    woM = np.asarray(wo, np.float32)      # [out 2048, in 2048]

    # global repartitioned feature f = 512 s + 128 g + 64 hh + d
    # <-> model head 8 s + 4 hh + g, dim d
    colmap4 = np.array([
        64 * (8 * s + 4 * hh + g) + d
        for s in range(4) for g in range(4) for hh in range(2)
        for d in range(64)
    ])
    W = woM[:, colmap4].T  # [in(f) 2048, out 2048]
    wo4 = b(
        W.reshape(4, 4, 128, 16, 128).transpose(2, 1, 3, 0, 4)
    )  # [rho_in 128, g 4, n 16, s 4, rho_out 128]

    in_maps = []
    for c in range(N_CORES):
        bc, pp = c // 4, c % 4
        xbT = np.asarray(x[bc], np.float32).T
        xq4 = b(xbT.reshape(KT, 128, 4, 512).transpose(1, 2, 0, 3))
        # wq: pair pp, slot g holds heads 8pp+4hh+g at rows 64hh+d
        qcols = np.array([
            64 * (8 * pp + 4 * hh + g) + d
            for g in range(4) for hh in range(2) for d in range(64)
        ])
        wq4 = b(
            wqT[:, qcols].reshape(KT, 128, 4, 128).transpose(1, 2, 0, 3)
        )
        kcols = np.array([
            64 * (2 * pp + hh) + d for hh in range(2) for d in range(64)
        ])
        wk4 = b(wkT[:, kcols].reshape(KT, 128, 128).transpose(1, 0, 2))
        wv4 = b(wvT[:, kcols].reshape(KT, 128, 128).transpose(1, 0, 2))
        bsel_c = np.zeros((128, 2), np.float32)
        bsel_c[:, 0 if bc == 0 else 1] = 1.0
        in_maps.append(
            {
                "xq4": xq4, "wq4": wq4, "wk4": wk4, "wv4": wv4, "wo4": wo4,
                "cosK": cosK_b, "sinK": sinK_b, "pshuf": pshuf_b,
                "cmask4": cmask_b, "selA": selA_b, "selB": selB_b,
                "bsel": bsel_c,
            }
        )
    return in_maps


_NC_CACHE = {}


def get_nc():
    if "hp" not in _NC_CACHE:
        _NC_CACHE["hp"] = build_gqa_hp()
    return _NC_CACHE["hp"]


def kernel(x, cos, sin, wq, wk, wv, wo, _trace=False):
    x = np.asarray(x, np.float32)
    nc = get_nc()
    in_maps = make_inputs(
        x,
        np.asarray(cos, np.float32),
        np.asarray(sin, np.float32),
        np.asarray(wq, np.float32),
        np.asarray(wk, np.float32),
        np.asarray(wv, np.float32),
        np.asarray(wo, np.float32),
    )
    res = run_bass_kernel_spmd(nc, in_maps, list(range(N_CORES)), trace=_trace)
    # core c returns out^T [2048, 512] (bf16) for batch c//4, tokens
    # [512*(c%4), 512*(c%4)+512)
    full = np.empty((B, T, DM), np.float32)
    for c in range(N_CORES):
        bc, pp = c // 4, c % 4
        full[bc, 512 * pp:512 * (pp + 1), :] = np.asarray(
            res.results[c]["out"], np.float32
        ).T
    if _trace:
        return full, res
    return full


# revision 44
# speedup vs baseline: 1.1020x; 1.0333x over previous
"""GQA attention (B=2, T=2048, d_model=2048, 32 Q heads, 8 KV heads,
head_dim=64, RoPE, causal) on 8 Trainium2 NeuronCores -- head-parallel.

Sharding v4: core c = (batch c//4, kv-pair p=c%4). Each core projects
K/V for its ONE kv-pair (2 kv heads) over the 2048 tokens of its batch,
Q for the pair's 8 query heads, and runs fully-causal attention for all
2048 queries: exp/score element count is the exact causal half -- ~1.9x
fewer exp elements than the sequence-parallel v3 layout, which was
ACT-bound. Attention output y (512 features x 2048 tokens per core) is
repartitioned token-wise via AllToAll collectives within each 4-core
batch group; each core then applies the full output projection for its
512 tokens, emitting out^T [2048, 512] exactly like v3 (host unshard
unchanged).

The A2A is split by head slot (g0+g1 after round 1, g2 after round 2,
g3 after round 3): the first instance absorbs launch skew under two
attention rounds; the last moves only 0.5MB and its output-projection
quarter is the only serial tail. Output projection accumulates in SBUF
across the three arrivals. Q projection for g=1,2,3 is deferred into
rounds 0-2 (re-streamed x half-tiles) so the PE stays fed while ACT
grinds the exp stream."""

import sys

for _p in ("/opt/trn_rl_repo",):
    if _p not in sys.path:
        sys.path.insert(0, _p)

from contextlib import ExitStack

import numpy as np

import concourse.bass as bass  # noqa: F401
import concourse.mybir as mybir
import concourse.tile as tile
from concourse import bacc
from concourse.bass_utils import run_bass_kernel_spmd

F32 = mybir.dt.float32
BF16 = mybir.dt.bfloat16

B = 2
T = 2048
DM = 2048
HD = 64
N_CORES = 8
KT = DM // 128
NKB = T // 128
SCALE = 1.0 / float(np.sqrt(HD))


def build_gqa_hp():
    nc = bacc.Bacc(
        "TRN2", target_bir_lowering=False, debug=False, num_devices=N_CORES
    )

    xq4 = nc.dram_tensor("xq4", [128, 4, KT, 512], BF16, kind="ExternalInput")
    wq4 = nc.dram_tensor("wq4", [128, 4, KT, 128], BF16, kind="ExternalInput")
    wk4 = nc.dram_tensor("wk4", [128, KT, 128], BF16, kind="ExternalInput")
    wv4 = nc.dram_tensor("wv4", [128, KT, 128], BF16, kind="ExternalInput")
    wo4 = nc.dram_tensor("wo4", [128, 4, 16, 4, 128], BF16,
                         kind="ExternalInput")
    cosK = nc.dram_tensor("cosK", [128, T], BF16, kind="ExternalInput")
    sinK = nc.dram_tensor("sinK", [128, T], BF16, kind="ExternalInput")
    pshuf = nc.dram_tensor("pshuf", [128, 128], BF16, kind="ExternalInput")
    cmask4 = nc.dram_tensor("cmask4", [128, 4, 512], BF16,
                            kind="ExternalInput")
    selA = nc.dram_tensor("selA", [128, 4, 128], BF16, kind="ExternalInput")
    selB = nc.dram_tensor("selB", [128, 4, 128], BF16, kind="ExternalInput")
    out = nc.dram_tensor("out", [DM, 512], BF16, kind="ExternalOutput")

    # A2A buffers (8-rank; 4-rank meshes unsupported): dram part
    # d = 16*shard + s5; per-part free [rholo(8), g(n), tok(512)];
    # sbuf feature row rho = 8*s5 + rholo. Shard j carries the local
    # token block j%4, duplicated into both group halves so the same
    # SPMD program serves both batch groups; receivers pick their
    # half with a per-core 0/1 selector (bsel).
    cc_in = [
        nc.dram_tensor(f"cc_in{i}", [128, 8, 1, 512], BF16)
        for i in range(4)
    ]
    cc_out = [
        nc.dram_tensor(f"cc_out{i}", [128, 8, 1, 512], BF16)
        for i in range(4)
    ]
    CC_GS = [(0,), (1,), (2,), (3,)]
    GROUPS = [[0, 1, 2, 3, 4, 5, 6, 7]]
    bseli = nc.dram_tensor("bseli", [1, 2], mybir.dt.int32,
                           kind="ExternalInput")

    Exp = mybir.ActivationFunctionType.Exp
    Ln = mybir.ActivationFunctionType.Ln

    with tile.TileContext(nc) as tc, ExitStack() as ctx:
        PER = ctx.enter_context(tc.tile_pool(name="per", bufs=1))
        EXPP = ctx.enter_context(tc.tile_pool(name="expp", bufs=1))
        PS = ctx.enter_context(tc.tile_pool(name="ps", bufs=2, space="PSUM"))
        p1ctx = ExitStack()
        P1 = p1ctx.enter_context(tc.tile_pool(name="p1", bufs=1))

        wk_sb = PER.tile([128, KT, 128], BF16, tag="wk")
        wv_sb = PER.tile([128, KT, 128], BF16, tag="wv")
        wq_sb = PER.tile([128, 4, KT, 128], BF16, tag="wq")
        cosK_sb = PER.tile([128, T], BF16, tag="cosK")
        sinK_sb = PER.tile([128, T], BF16, tag="sinK")
        pshuf_sb = PER.tile([128, 128], BF16, tag="pshuf")
        selA_sb = PER.tile([128, 4, 128], BF16, tag="selA")
        selB_sb = PER.tile([128, 4, 128], BF16, tag="selB")
        cmask_sb = PER.tile([128, 4, 512], BF16, tag="cmask4")
        nc.gpsimd.dma_start(out=pshuf_sb, in_=pshuf.ap())

        qrp = PER.tile([128, 4, T], BF16, tag="qrp")
        ktp = PER.tile([128, T], BF16, tag="ktp")
        vaug = PER.tile([128, NKB, 2, HD + 1], BF16, tag="vaug")
        nc.gpsimd.memset(vaug[:, :, :, HD:HD + 1], 1.0)
        yt = PER.tile([128, 4, T], BF16, tag="yt")
        denA = PER.tile([128, 2, T], BF16, tag="den")
        nc.gpsimd.memset(denA, 1.0)

        def rope_pair(dst, src_ps, cos_s, sin_s, pool):
            s_sb = pool.tile([128, 512], BF16, tag="rp_s", bufs=3,
                             name="rp_s")
            nc.vector.tensor_copy(s_sb, src_ps)
            sh_ps = PS.tile([128, 512], F32, tag="pv")
            nc.tensor.matmul(sh_ps, pshuf_sb, s_sb, start=True, stop=True)
            t1 = pool.tile([128, 512], BF16, tag="rp_t1", bufs=3, name="rp_t1")
            nc.vector.tensor_mul(t1, s_sb, cos_s)
            t2 = pool.tile([128, 512], BF16, tag="rp_t2", bufs=3, name="rp_t2")
            nc.vector.tensor_mul(t2, sh_ps, sin_s)
            nc.vector.tensor_add(dst, t1, t2)

        # ---- phase 1: K, V, Q(g=0) projections + RoPE (m-tile order)
        first = True
        for mi in range(4):
            xt = P1.tile([128, KT, 512], BF16, tag="xt", bufs=2)
            for kg in range(4):
                nc.sync.dma_start(
                    out=xt[:, 4 * kg:4 * (kg + 1), :],
                    in_=xq4.ap()[:, mi, 4 * kg:4 * (kg + 1), :],
                )
            if first:
                nc.scalar.dma_start(out=wk_sb, in_=wk4.ap())
                nc.scalar.dma_start(out=wq_sb[:, 0], in_=wq4.ap()[:, 0])
                nc.sync.dma_start(out=cosK_sb, in_=cosK.ap())
                nc.sync.dma_start(out=sinK_sb, in_=sinK.ap())
                nc.scalar.dma_start(out=wv_sb, in_=wv4.ap())
                for g in range(1, 4):
                    nc.scalar.dma_start(out=wq_sb[:, g], in_=wq4.ap()[:, g])
                nc.sync.dma_start(out=cmask_sb, in_=cmask4.ap())
                nc.sync.dma_start(out=selA_sb, in_=selA.ap())
                nc.sync.dma_start(out=selB_sb, in_=selB.ap())
                first = False
            ms = 512 * mi
            kp = PS.tile([128, 512], F32, tag="pA")
            for kt in range(KT):
                nc.tensor.matmul(
                    kp, wk_sb[:, kt, :], xt[:, kt, :],
                    start=(kt == 0), stop=(kt == KT - 1),
                )
            rope_pair(
                ktp[:, ms:ms + 512], kp,
                cosK_sb[:, ms:ms + 512], sinK_sb[:, ms:ms + 512], P1,
            )
            for g in (0, 3):
                qp = PS.tile([128, 512], F32, tag="pA")
                for kt in range(KT):
                    nc.tensor.matmul(
                        qp, wq_sb[:, g, kt, :], xt[:, kt, :],
                        start=(kt == 0), stop=(kt == KT - 1),
                    )
                rope_pair(
                    qrp[:, g, ms:ms + 512], qp,
                    cosK_sb[:, ms:ms + 512], sinK_sb[:, ms:ms + 512], P1,
                )
            for j in range(4):
                kb = 4 * mi + j
                vp = PS.tile([128, 128], F32, tag="pv")
                for kt in range(KT):
                    nc.tensor.matmul(
                        vp, xt[:, kt, 128 * j:128 * (j + 1)], wv_sb[:, kt, :],
                        start=(kt == 0), stop=(kt == KT - 1),
                    )
                nc.scalar.copy(vaug[:, kb, :, 0:HD], vp)
        p1ctx.close()

        p3ctx = ExitStack()
        PX = p3ctx.enter_context(tc.tile_pool(name="px", bufs=1))
        wo_sb = PX.tile([128, 2, 16, 4, 128], BF16, tag="wo")
        oacc = PX.tile([128, 16, 512], BF16, tag="oacc")
        yF = PX.tile([128, 4, 4, 512], BF16, tag="yF")
        _r0 = nc.sync.alloc_register("selr0")
        nc.sync.reg_load(_r0, bseli[0:1, 0:1])
        selr0 = nc.sync.snap(_r0, donate=True, min_val=0, max_val=1)
        _r1 = nc.sync.alloc_register("selr1")
        nc.sync.reg_load(_r1, bseli[0:1, 1:2])
        selr1 = nc.sync.snap(_r1, donate=True, min_val=0, max_val=1)

        # ---- deferred Q projection (g=1..3 inside rounds g-1)
        xh_q = []

        def q_prefetch(g):
            for mi in range(4):
                for h in range(2):
                    xh = PX.tile([128, 8, 512], BF16, tag="xh", bufs=2,
                                 name="xh")
                    nc.sync.dma_start(
                        out=xh, in_=xq4.ap()[:, mi, 8 * h:8 * h + 8, :]
                    )
                    xh_q.append((g, mi, h, xh))

        def q_proj_deferred(n_mi):
            for _ in range(n_mi):
                qp = PS.tile([128, 512], F32, tag="pA", name="qpd")
                for hh in range(2):
                    g, mi, h, xh = xh_q.pop(0)
                    for kt in range(8):
                        nc.tensor.matmul(
                            qp, wq_sb[:, g, 8 * h + kt, :], xh[:, kt, :],
                            start=(h == 0 and kt == 0),
                            stop=(h == 1 and kt == 7),
                        )
                ms = 512 * mi
                rope_pair(
                    qrp[:, g, ms:ms + 512], qp,
                    cosK_sb[:, ms:ms + 512], sinK_sb[:, ms:ms + 512], PX,
                )

        def normalize(g, qp):
            # broadcast den to 128 partitions FIRST (the sel matmuls),
            # then one full-width DVE reciprocal: no ACT involvement at
            # all -- the old ln/exp pair cost two ACT table-set switches
            # (~2.6us) plus 2x2us of 32-partition ACT per call, right on
            # the round-boundary critical path
            qs = 1024 * qp
            base = 32 * g
            kw = {"tile_position": (96, 0)} if g == 3 else {}
            for h in range(2):
                hs = 512 * h
                rb_ps = PS.tile([128, 512], F32, tag="pA")
                nc.tensor.matmul(
                    rb_ps, selA_sb[base:base + 32, g, :],
                    denA[base:base + 32, 0, qs + hs:qs + hs + 512],
                    start=True, stop=False, **kw,
                )
                nc.tensor.matmul(
                    rb_ps, selB_sb[base:base + 32, g, :],
                    denA[base:base + 32, 1, qs + hs:qs + hs + 512],
                    start=False, stop=True, **kw,
                )
                rcp = PX.tile([128, 512], F32, tag="rcp", bufs=2)
                nc.vector.reciprocal(rcp, rb_ps)
                nc.vector.tensor_mul(
                    yt[:, g, qs + hs:qs + hs + 512],
                    yt[:, g, qs + hs:qs + hs + 512], rcp
                )

        def attention_round(g, qcs, filler=None):
            for qc in qcs:
                qs = 512 * qc
                pv = [
                    PS.tile([HD + 1, 512], F32, tag="pv",
                            name=f"pv{g}{qc}{hh}")
                    for hh in range(2)
                ]
                nkb = 4 * qc + 4
                for kb in range(nkb):
                    ql = 128 * max(0, kb - 4 * qc)
                    s2 = PS.tile([128, 2, 512], F32, tag="s2", name="s2")
                    for hh in range(2):
                        nc.tensor.matmul(
                            s2[:, hh, ql:],
                            ktp[64 * hh:64 * (hh + 1),
                                128 * kb:128 * (kb + 1)],
                            qrp[64 * hh:64 * (hh + 1), g, qs + ql:qs + 512],
                            start=True, stop=True,
                            tile_position=(64 * hh, 0),
                        )
                    e_sb = EXPP.tile([128, 2, 512], BF16, tag="e_sb", bufs=6)
                    nc.scalar.activation(
                        e_sb[:, :, ql:], s2[:, :, ql:], Exp, scale=SCALE,
                    )
                    di = kb - 4 * qc
                    if di >= 0:
                        for hh in range(2):
                            nc.vector.tensor_mul(
                                e_sb[:, hh, ql:], e_sb[:, hh, ql:],
                                cmask_sb[:, di, ql:],
                            )
                    for hh in range(2):
                        nc.tensor.matmul(
                            pv[hh][:, ql:], vaug[:, kb, hh, :],
                            e_sb[:, hh, ql:],
                            start=(kb == 0), stop=(kb == nkb - 1),
                            skip_group_check=(ql > 0),
                        )
                    if filler and kb % 2 == 1:
                        filler.pop(0)()
                for hh in range(2):
                    nc.vector.tensor_copy(
                        denA[32 * g:32 * g + 1, hh, qs:qs + 512],
                        pv[hh][HD:HD + 1, :],
                    )
                    nc.vector.tensor_copy(
                        yt[64 * hh:64 * (hh + 1), g, qs:qs + 512],
                        pv[hh][0:HD, :],
                    )
                if qc % 2 == 1:
                    normalize(g, qc // 2)

        # ---- collectives (manual sems: DRAM deps aren't tile-tracked)

        def write_cc(idx, ts):
            gs = CC_GS[idx]
            for t in ts:
                for rep in (t, t + 4):
                    nc.gpsimd.dma_start(
                        out=cc_in[idx].ap()[16 * rep:16 * (rep + 1)],
                        in_=yt[:, gs[0]:gs[-1] + 1, 512 * t:512 * (t + 1)],
                    )

        def emit_a2a(idx, ts=(0, 1, 2, 3)):
            write_cc(idx, ts)
            nc.gpsimd.collective_compute(
                "AllToAll",
                mybir.AluOpType.bypass,
                replica_groups=GROUPS,
                ins=[cc_in[idx].ap().opt()],
                outs=[cc_out[idx].ap().opt()],
            )

        def load_wo_slab(g):
            # ACT queue: idle in the tail; gpsimd carries the collective
            # chain and would serialize these behind the A2A launches
            nc.scalar.dma_start(out=wo_sb[:, g % 2], in_=wo4.ap()[:, g])

        def fetch_yF(idx):
            # conditional DMAs select the core's batch-group half at the
            # transfer level: no elementwise selects, so nothing can
            # head-of-line-block the DVE queue on the collective wait
            g = CC_GS[idx][0]
            for s in range(4):
                nc.sync.dma_start(
                    out=yF[:, g, s, :],
                    in_=cc_out[idx].ap()[16 * s:16 * (s + 1), :, 0, :],
                    cond=selr0,
                )
                nc.sync.dma_start(
                    out=yF[:, g, s, :],
                    in_=cc_out[idx].ap()[16 * (s + 4):16 * (s + 5), :, 0, :],
                    cond=selr1,
                )

        def wo_chain_n(g, n, mode):
            op = PS.tile([128, 512], F32, tag="pA")
            for s in range(4):
                nc.tensor.matmul(
                    op, wo_sb[:, g % 2, n, s, :], yF[:, g, s, :],
                    start=(s == 0), stop=(s == 3),
                )
            if mode == 0:
                nc.vector.tensor_copy(oacc[:, n, :], op)
            elif mode == 1:
                nc.vector.tensor_add(oacc[:, n, :], oacc[:, n, :], op)
            else:
                ot = PX.tile([128, 512], BF16, tag="ot", bufs=2)
                nc.vector.tensor_add(ot, oacc[:, n, :], op)
                nc.sync.dma_start(
                    out=out.ap()[128 * n:128 * (n + 1), :], in_=ot
                )

        def wo_chain_n23(n):
            op = PS.tile([128, 512], F32, tag="pA")
            for gi, g in enumerate((2, 3)):
                for s in range(4):
                    nc.tensor.matmul(
                        op, wo_sb[:, g % 2, n, s, :], yF[:, g, s, :],
                        start=(gi == 0 and s == 0), stop=(gi == 1 and s == 3),
                    )
            ot = PX.tile([128, 512], BF16, tag="ot", bufs=2)
            nc.vector.tensor_add(ot, oacc[:, n, :], op)
            nc.sync.dma_start(
                out=out.ap()[128 * n:128 * (n + 1), :], in_=ot
            )

        def wo_chain(g, mode):
            """One output-projection pass: head-slot g's 4 F-tiles into
            all 16 out-row blocks. mode 0 = init oacc, 1 = accumulate,
            2 = final add + store."""
            for n in range(16):
                wo_chain_n(g, n, mode)

        # ---- rounds with deferred work woven in
        q_prefetch(1)
        for qc in range(4):
            attention_round(0, (qc,))
            q_proj_deferred(1)
        emit_a2a(0)
        q_prefetch(2)
        for qc in range(4):
            attention_round(1, (qc,))
            q_proj_deferred(1)
        emit_a2a(1)
        attention_round(2, (0, 1))
        attention_round(2, (2, 3))
        emit_a2a(2)
        load_wo_slab(0)
        load_wo_slab(1)
        attention_round(3, (0, 1))
        write_cc(3, (0, 1))
        fetch_yF(0)
        fetch_yF(1)
        wo01_fill = [
            (lambda n=n: wo_chain_n(0, n, 0)) for n in range(16)
        ] + [
            (lambda n=n: wo_chain_n(1, n, 1)) for n in range(16)
        ]
        attention_round(3, (2, 3), filler=wo01_fill)
        emit_a2a(3, ts=(2, 3))
        for th in wo01_fill:
            th()
        load_wo_slab(2)
        fetch_yF(2)
        wo_chain(2, 1)
        load_wo_slab(3)
        fetch_yF(3)
        wo_chain(3, 2)
        p3ctx.close()

    nc.finalize()
    return nc


def make_inputs(x, cos, sin, wq, wk, wv, wo):
    """Host-side sharding/layout prep. Returns in_maps for the 8 cores."""
    import ml_dtypes

    bf = ml_dtypes.bfloat16

    def b(arr):
        return np.ascontiguousarray(np.asarray(arr, dtype=bf))

    sgn = np.concatenate(
        [-np.ones(32, np.float32), np.ones(32, np.float32)]
    )
    pshuf = np.zeros((128, 128), np.float32)
    for m in range(128):
        pshuf[64 * (m // 64) + (m % 64 + 32) % 64, m] = 1.0
    selA = np.zeros((128, 4, 128), np.float32)
    selB = np.zeros((128, 4, 128), np.float32)
    for g in range(4):
        selA[32 * g, g, 0:64] = 1.0
        selB[32 * g, g, 64:128] = 1.0
    p = np.arange(128)[:, None]
    q = np.arange(512)[None, :]
    cmask4 = np.stack(
        [(128 * r + p <= q).astype(np.float32) for r in range(4)]
    ).transpose(1, 0, 2)  # [128, 4, 512]
    pshuf_b, selA_b, selB_b, cmask_b = b(pshuf), b(selA), b(selB), b(cmask4)

    cosK_b = b(np.tile(np.asarray(cos, np.float32).T, (2, 1)))
    sinK_b = b(np.tile(np.asarray(sin, np.float32).T * sgn[:, None], (2, 1)))

    wqT = np.asarray(wq, np.float32).T    # [in 2048, out 2048]
    wkT = np.asarray(wk, np.float32).T    # [in 2048, out 512]
    wvT = np.asarray(wv, np.float32).T
    woM = np.asarray(wo, np.float32)      # [out 2048, in 2048]

    # global repartitioned feature f = 512 s + 128 g + 64 hh + d
    # <-> model head 8 s + 4 hh + g, dim d
    colmap4 = np.array([
        64 * (8 * s + 4 * hh + g) + d
        for s in range(4) for g in range(4) for hh in range(2)
        for d in range(64)
    ])
    W = woM[:, colmap4].T  # [in(f) 2048, out 2048]
    wo4 = b(
        W.reshape(4, 4, 128, 16, 128).transpose(2, 1, 3, 0, 4)
    )  # [rho_in 128, g 4, n 16, s 4, rho_out 128]

    in_maps = []
    for c in range(N_CORES):
        bc, pp = c // 4, c % 4
        xbT = np.asarray(x[bc], np.float32).T
        xq4 = b(xbT.reshape(KT, 128, 4, 512).transpose(1, 2, 0, 3))
        # wq: pair pp, slot g holds heads 8pp+4hh+g at rows 64hh+d
        qcols = np.array([
            64 * (8 * pp + 4 * hh + g) + d
            for g in range(4) for hh in range(2) for d in range(64)
        ])
        wq4 = b(
            wqT[:, qcols].reshape(KT, 128, 4, 128).transpose(1, 2, 0, 3)
        )
        kcols = np.array([
            64 * (2 * pp + hh) + d for hh in range(2) for d in range(64)
        ])
        wk4 = b(wkT[:, kcols].reshape(KT, 128, 128).transpose(1, 0, 2))
        wv4 = b(wvT[:, kcols].reshape(KT, 128, 128).transpose(1, 0, 2))
        bseli_c = np.array(
            [[1, 0]] if bc == 0 else [[0, 1]], np.int32
        )
        in_maps.append(
            {
                "xq4": xq4, "wq4": wq4, "wk4": wk4, "wv4": wv4, "wo4": wo4,
                "cosK": cosK_b, "sinK": sinK_b, "pshuf": pshuf_b,
                "cmask4": cmask_b, "selA": selA_b, "selB": selB_b,
                "bseli": bseli_c,
            }
        )
    return in_maps


_NC_CACHE = {}


def get_nc():
    if "hp" not in _NC_CACHE:
        _NC_CACHE["hp"] = build_gqa_hp()
    return _NC_CACHE["hp"]


def kernel(x, cos, sin, wq, wk, wv, wo, _trace=False):
    x = np.asarray(x, np.float32)
    nc = get_nc()
    in_maps = make_inputs(
        x,
        np.asarray(cos, np.float32),
        np.asarray(sin, np.float32),
        np.asarray(wq, np.float32),
        np.asarray(wk, np.float32),
        np.asarray(wv, np.float32),
        np.asarray(wo, np.float32),
    )
    res = run_bass_kernel_spmd(nc, in_maps, list(range(N_CORES)), trace=_trace)
    # core c returns out^T [2048, 512] (bf16) for batch c//4, tokens
    # [512*(c%4), 512*(c%4)+512)
    full = np.empty((B, T, DM), np.float32)
    for c in range(N_CORES):
        bc, pp = c // 4, c % 4
        full[bc, 512 * pp:512 * (pp + 1), :] = np.asarray(
            res.results[c]["out"], np.float32
        ).T
    if _trace:
        return full, res
    return full
